# revision 13
# baseline (speedup 1.0000x reference)
"""GAT (3-layer, 4-head) + mean-pool + MLP on 8 Trainium2 NeuronCores.

Strategy (see spec sharding_hint):
  - Nodes sharded 8 ways by destination; each core owns N/8 dst nodes and all
    edges incident to them (1-D graph partition, built on host from the index
    tensors only).
  - Dense phase replicated: every core computes the full node table
    T[n] = [h(256) | alpha_src(4) | alpha_dst(4)] in bf16 via matmuls whose
    rhs carries two extra score columns (Ws = W @ blockdiag(a_s), Wd likewise).
  - Edge phase: per 128-edge chunk, dma_gather h/alpha rows by src (and alpha
    rows by dst), compute al = exp(leaky_relu(as+ad)) (softmax shift skipped --
    mathematically invariant), then scatter-accumulate al*h into a PSUM window
    of 127 dst slots via a selection-matrix matmul; the matmul also sums al
    into 4 extra columns, yielding the softmax denominators for free.
  - Between layers: AllGather of the (transposed) activations; final graph
    mean-pool via indicator matmul accumulated across windows + AllReduce.
"""
import math
import numpy as np
from contextlib import ExitStack

import concourse.bass as bass
import concourse.bacc as bacc
import concourse.mybir as mybir
import concourse.tile as tile
from concourse import library_config
from concourse.bass_utils import run_bass_kernel_spmd

F32 = mybir.dt.float32
BF16 = mybir.dt.bfloat16
I16 = mybir.dt.int16
NPBF16 = mybir.dt.np(BF16)

P = 128
WIN = 127          # real dst slots per PSUM window; slot 127 collects padding
NCORES = 8
H, C = 4, 64
HC = H * C         # 256
NEG = 0.2
TSTRIDE = 384      # table row stride (elements); bf16 -> 768B (%256 == 0)

AF = mybir.ActivationFunctionType
OP = mybir.AluOpType


# ----------------------------------------------------------------- host prep

def _wrap_idx(idx_flat):
    """[num] -> [128, num//16] int16 tile layout: index j at [j%16, j//16],
    replicated 8x along partitions (one stripe per Q7 core)."""
    num = idx_flat.shape[0]
    w = idx_flat.reshape(num // 16, 16).T.astype(np.int16)   # [16, num//16]
    return np.tile(w, (8, 1))


def _prep_core(k, src, dst, batch, N, NLOC, NWIN, G):
    """Per-core edge structure. Returns dict of window-major arrays + counts."""
    sel = (dst >= k * NLOC) & (dst < (k + 1) * NLOC)
    s = src[sel].astype(np.int64)
    dl = (dst[sel] - k * NLOC).astype(np.int64)
    order = np.argsort(dl, kind="stable")
    s, dl = s[order], dl[order]
    w = dl // WIN
    slot = dl - w * WIN
    counts = np.bincount(w, minlength=NWIN)
    return s, w, slot, counts


def _build_host_data(edge_index, batch, N, G):
    NLOC = N // NCORES
    NWIN = math.ceil(NLOC / WIN)
    src = np.concatenate([np.asarray(edge_index[0]), np.arange(N)])
    dst = np.concatenate([np.asarray(edge_index[1]), np.arange(N)])
    batch = np.asarray(batch)

    per_core = [_prep_core(k, src, dst, batch, N, NLOC, NWIN, G)
                for k in range(NCORES)]
    NCW = max(int(math.ceil(c / P)) for (_, _, _, cnts) in per_core
              for c in cnts)
    NE = NCW * P  # padded edges per window

    data = []
    for k in range(NCORES):
        s, w, slot, counts = per_core[k]
        hsrc = np.zeros((NWIN, NE), np.int64)          # gather-by-src rows
        sdst = np.zeros((NWIN, NE), np.int64)          # gather-by-dst rows
        slots = np.full((NWIN, NE), WIN, np.int64)     # pad -> trash slot 127
        off = np.concatenate([[0], np.cumsum(counts)])
        for wi in range(NWIN):
            cnt = counts[wi]
            sl = slice(off[wi], off[wi] + cnt)
            hsrc[wi, :cnt] = s[sl]
            sdst[wi, :cnt] = k * NLOC + wi * WIN + slot[sl]
            slots[wi, :cnt] = slot[sl]
        hidx = np.stack([_wrap_idx(hsrc[wi]) for wi in range(NWIN)])
        sidx = np.stack([_wrap_idx(sdst[wi]) for wi in range(NWIN)])
        # edge i of a window sits at [i%128, i//128] in gathered tiles
        dslot = np.stack([slots[wi].reshape(NCW, P).T for wi in range(NWIN)])
        bind = np.zeros((NWIN, P, G), np.float32)
        for wi in range(NWIN):
            base = k * NLOC + wi * WIN
            nreal = min(WIN, (k + 1) * NLOC - base)
            rows = np.arange(nreal)
            bind[wi, rows, batch[base + rows]] = 1.0
        data.append(dict(
            hidx=hidx.astype(np.int16),
            sidx=sidx.astype(np.int16),
            dslot=dslot.astype(np.float32),
            bind=bind.astype(NPBF16),
        ))
    return data, NLOC, NWIN, NCW


# ------------------------------------------------------------ program build

def _strided_view(t_ap, offset, stride, rows, width):
    """AP over DRAM table: rows x width, row-stride `stride`, elem offset."""
    flat = t_ap.rearrange("v s -> (v s)")
    flat = flat[offset:offset + rows * stride]
    v = flat.rearrange("(v s) -> v s", s=stride)
    return v[:, 0:width]


def build_program(N, NLOC, NWIN, NCW, G, FIN0):
    """Builds the full 3-layer SPMD program. Same program for all cores."""
    NT = N + 1           # table rows (+1 pad row for overrunning score reads)
    NE = NCW * P
    NBLK = NCORES        # node blocks (one per core's node range)
    NTIL = math.ceil(NLOC / P)   # dense tiles per block
    fins = [FIN0, HC, HC]

    nc = bacc.Bacc()

    # ---------- parameters
    pr = {}
    for i, fin in enumerate(fins):
        pr[f'W{i}'] = nc.declare_dram_parameter(f'W{i}', [fin, HC], F32, isOutput=False)
        pr[f'as{i}'] = nc.declare_dram_parameter(f'as{i}', [H, C], F32, isOutput=False)
        pr[f'ad{i}'] = nc.declare_dram_parameter(f'ad{i}', [H, C], F32, isOutput=False)
        pr[f'b{i}'] = nc.declare_dram_parameter(f'b{i}', [HC], F32, isOutput=False)
    pr['Wm1'] = nc.declare_dram_parameter('Wm1', [HC, C], F32, isOutput=False)
    pr['bm1'] = nc.declare_dram_parameter('bm1', [C], F32, isOutput=False)
    pr['Wm2'] = nc.declare_dram_parameter('Wm2', [C, 2], F32, isOutput=False)
    pr['bm2'] = nc.declare_dram_parameter('bm2', [2], F32, isOutput=False)
    x0tb = nc.declare_dram_parameter('x0tb', [NBLK, FIN0, NLOC], F32, isOutput=False)
    hidx_in = nc.declare_dram_parameter('hidx', [NWIN, P, NE // 16], I16, isOutput=False)
    sidx_in = nc.declare_dram_parameter('sidx', [NWIN, P, NE // 16], I16, isOutput=False)
    dslot_in = nc.declare_dram_parameter('dslot', [NWIN, P, NCW], F32, isOutput=False)
    bind_in = nc.declare_dram_parameter('bind', [NWIN, P, G], BF16, isOutput=False)
    iota_in = nc.declare_dram_parameter('iota', [P, P], BF16, isOutput=False)
    idf_in = nc.declare_dram_parameter('identf', [P, P], F32, isOutput=False)
    idb_in = nc.declare_dram_parameter('identb', [P, P], BF16, isOutput=False)
    mblk_in = nc.declare_dram_parameter('maskblk', [P, 2, H], F32, isOutput=False)
    out_p = nc.declare_dram_parameter('out', [G, 2], F32, isOutput=True)

    # ---------- internal DRAM
    T = nc.dram_tensor('Tbl', [NT, TSTRIDE], BF16)
    xT_own = nc.dram_tensor('xT_own', [HC, NLOC], BF16)
    xT_all = nc.dram_tensor('xT_all', [NBLK, HC, NLOC], BF16, addr_space="Shared")
    pool_in = nc.dram_tensor('pool_in', [G, HC + 1], F32)
    pool_out = nc.dram_tensor('pool_out', [G, HC + 1], F32, addr_space="Shared")

    with ExitStack() as ctx:
        tc = ctx.enter_context(tile.TileContext(nc))
        cst = ctx.enter_context(tc.tile_pool(name="cst", bufs=1))
        dns = ctx.enter_context(tc.tile_pool(name="dns", bufs=2))
        wts = ctx.enter_context(tc.tile_pool(name="wts", bufs=2))
        edg = ctx.enter_context(tc.tile_pool(name="edg", bufs=2))
        sml = ctx.enter_context(tc.tile_pool(name="sml", bufs=4))
        fin_pool = ctx.enter_context(tc.tile_pool(name="fin", bufs=2))
        pd = ctx.enter_context(tc.tile_pool(name="pd", bufs=2, space="PSUM"))
        pa = ctx.enter_context(tc.tile_pool(name="pa", bufs=2, space="PSUM"))
        pt = ctx.enter_context(tc.tile_pool(name="pt", bufs=2, space="PSUM"))
        pp = ctx.enter_context(tc.tile_pool(name="pp", bufs=1, space="PSUM"))

        # constants
        iota_t = cst.tile([P, P], BF16)
        nc.sync.dma_start(out=iota_t[:], in_=iota_in[:])
        idf_t = cst.tile([P, P], F32)
        nc.sync.dma_start(out=idf_t[:], in_=idf_in[:])
        idb_t = cst.tile([P, P], BF16)
        nc.sync.dma_start(out=idb_t[:], in_=idb_in[:])
        mblk_t = cst.tile([P, 2, H], F32)
        nc.sync.dma_start(out=mblk_t[:], in_=mblk_in[:])

        # zero the table once: gathers read full rows incl. pad columns
        zt = dns.tile([P, TSTRIDE], BF16, tag="zt")
        nc.vector.memset(zt[:], 0.0)
        for r0 in range(0, NT, P):
            wd = min(P, NT - r0)
            nc.sync.dma_start(out=T[r0:r0 + wd, :], in_=zt[0:wd, :])

        T_h_view = T[:]                                       # rows for h-gather
        T_s_view = _strided_view(T[:], HC, TSTRIDE, NT - 1, P)  # score rows

        pool_ps = pp.tile([G, HC + 1], F32)

        for li, fin in enumerate(fins):
            nk = fin // P    # k-tiles (1 or 2)

            # ---- per-layer weight prep: Wc[kt] = [W | Ws | Wd] in bf16
            wt = []
            for kt in range(nk):
                w_t = wts.tile([P, HC], F32, tag="wld")
                nc.sync.dma_start(out=w_t[:], in_=pr[f'W{li}'][kt * P:(kt + 1) * P, :])
                wt.append(w_t)
            # W^T tiles: WT[cb] = [128 (c-block), fin]
            WT = []
            for cb in range(2):
                wT_t = wts.tile([P, fin], F32, tag="wT")
                for kt in range(nk):
                    tp = pt.tile([P, P], F32, tag="tp")
                    nc.tensor.transpose(out=tp[:], in_=wt[kt][:, cb * P:(cb + 1) * P],
                                        identity=idf_t[:])
                    nc.vector.tensor_copy(out=wT_t[:, kt * P:(kt + 1) * P], in_=tp[:])
                WT.append(wT_t)
            # A-blocks for a_s / a_d  (A[p, h] = a[h, c]*[p//64 == h], p = 64h+c)
            ws_sb = []
            for which in ('as', 'ad'):
                a_flat = sml.tile([P, 1], F32, tag="aflat")
                acc = pt.tile([H, fin], F32, tag="tp")
                for cb in range(2):
                    nc.sync.dma_start(
                        out=a_flat[:],
                        in_=pr[f'{which}{li}'][:].rearrange("h c -> (h c)")[cb * P:(cb + 1) * P, None])
                    ab_t = sml.tile([P, H], F32, tag="ablk")
                    nc.vector.tensor_tensor(out=ab_t[:], in0=a_flat[:].to_broadcast([P, H]),
                                            in1=mblk_t[:, cb, :], op=OP.mult)
                    nc.tensor.matmul(out=acc[:], lhsT=ab_t[:], rhs=WT[cb][:],
                                     start=(cb == 0), stop=(cb == 1))
                wsT_sb = sml.tile([H, fin], F32, tag="wsTsb")
                nc.vector.tensor_copy(out=wsT_sb[:], in_=acc[:])
                ws_sb.append(wsT_sb)
            # assemble Wc tiles
            Wc = []
            for kt in range(nk):
                wc_t = wts.tile([P, HC + 2 * H], BF16, tag="wc")
                nc.vector.tensor_copy(out=wc_t[:, 0:HC], in_=wt[kt][:])
                for wi, wsT_sb in enumerate(ws_sb):
                    tp = pt.tile([P, H], F32, tag="tp")
                    nc.tensor.transpose(out=tp[:], in_=wsT_sb[:, kt * P:(kt + 1) * P],
                                        identity=idf_t[0:H, 0:H])
                    nc.vector.tensor_copy(
                        out=wc_t[:, HC + wi * H:HC + (wi + 1) * H], in_=tp[:])
                Wc.append(wc_t)
            # bias replicated
            b_row = sml.tile([1, HC], F32, tag="brow")
            nc.sync.dma_start(out=b_row[:], in_=pr[f'b{li}'][None, :])
            brep_t = cst.tile([P, HC], F32, tag="brep")
            nc.gpsimd.partition_broadcast(brep_t[:], b_row[:])

            # ---- dense phase: T[n] = [x @ Wc] for all n
            for blk in range(NBLK):
                xb = []
                for kt in range(nk):
                    xb_t = dns.tile([P, NLOC], BF16, tag=f"xb{kt}")
                    if li == 0:
                        nc.gpsimd.dma_start(out=xb_t[:], in_=x0tb[blk, kt * P:(kt + 1) * P, :])
                    else:
                        nc.sync.dma_start(out=xb_t[:], in_=xT_all[blk, kt * P:(kt + 1) * P, :])
                    xb.append(xb_t)
                for t in range(NTIL):
                    lo = t * P
                    wd = min(P, NLOC - lo)
                    ps_t = pd.tile([P, HC + 2 * H], F32, tag="pdense")
                    for kt in range(nk):
                        nc.tensor.matmul(out=ps_t[0:wd, :], lhsT=xb[kt][:, lo:lo + wd],
                                         rhs=Wc[kt][:], start=(kt == 0), stop=(kt == nk - 1))
                    tb_t = dns.tile([P, HC + 2 * H], BF16, tag="tb")
                    nc.scalar.copy(out=tb_t[0:wd, :], in_=ps_t[0:wd, :])
                    row0 = blk * NLOC + lo
                    nc.sync.dma_start(out=T[row0:row0 + wd, 0:HC + 2 * H], in_=tb_t[0:wd, :])

            # ---- edge phase (this core's windows)
            for w in range(NWIN):
                base = w * WIN
                wr = min(WIN, NLOC - base)
                hix = edg.tile([P, NE // 16], I16, tag="hix")
                nc.sync.dma_start(out=hix[:], in_=hidx_in[w])
                six = edg.tile([P, NE // 16], I16, tag="six")
                nc.sync.dma_start(out=six[:], in_=sidx_in[w])
                dsl = edg.tile([P, NCW], F32, tag="dsl")
                nc.sync.dma_start(out=dsl[:], in_=dslot_in[w])

                gh = edg.tile([P, NCW, TSTRIDE], BF16, tag="gh")
                nc.gpsimd.dma_gather(out_ap=gh[:], in_ap=T_h_view, idxs_ap=hix[:],
                                     num_idxs=NE, num_idxs_reg=NE,
                                     elem_size=TSTRIDE, elem_step=TSTRIDE,
                                     single_packet=False)
                gs = edg.tile([P, NCW, P], BF16, tag="gs")
                nc.gpsimd.dma_gather(out_ap=gs[:], in_ap=T_s_view, idxs_ap=six[:],
                                     num_idxs=NE, num_idxs_reg=NE,
                                     elem_size=P, elem_step=TSTRIDE,
                                     single_packet=False)

                agg = pa.tile([P, HC + H], F32, tag="agg")
                for c in range(NCW):
                    e_t = sml.tile([P, H], F32, tag="e")
                    nc.vector.tensor_tensor(out=e_t[:], in0=gh[:, c, HC:HC + H],
                                            in1=gs[:, c, H:2 * H], op=OP.add)
                    l_t = sml.tile([P, H], F32, tag="lrelu")
                    nc.vector.scalar_tensor_tensor(out=l_t[:], in0=e_t[:], scalar=NEG,
                                                   in1=e_t[:], op0=OP.mult, op1=OP.max)
                    rhs_t = sml.tile([P, HC + H], BF16, tag="rhs")
                    nc.scalar.activation(rhs_t[:, HC:HC + H], l_t[:], AF.Exp)
                    nc.vector.tensor_tensor(
                        out=rhs_t[:, 0:HC].rearrange("p (h c) -> p h c", h=H),
                        in0=gh[:, c, 0:HC].rearrange("p (h c) -> p h c", h=H),
                        in1=rhs_t[:, HC:HC + H, None].to_broadcast([P, H, C]),
                        op=OP.mult)
                    s_t = sml.tile([P, P], BF16, tag="S")
                    nc.vector.tensor_scalar(out=s_t[:], in0=iota_t[:],
                                            scalar1=dsl[:, c:c + 1], scalar2=None,
                                            op0=OP.is_equal)
                    nc.tensor.matmul(out=agg[:], lhsT=s_t[:], rhs=rhs_t[:],
                                     start=(c == 0), stop=(c == NCW - 1))

                # finalize window: out = elu(num/den + b)
                rec_t = sml.tile([P, H], F32, tag="rec")
                nc.vector.tensor_scalar(out=rec_t[:], in0=agg[:, HC:HC + H],
                                        scalar1=1e-30, scalar2=None, op0=OP.add)
                nc.vector.reciprocal(out=rec_t[:], in_=rec_t[:])
                sc_t = fin_pool.tile([P, HC], F32, tag="scaled")
                nc.vector.tensor_tensor(
                    out=sc_t[:].rearrange("p (h c) -> p h c", h=H),
                    in0=agg[:, 0:HC].rearrange("p (h c) -> p h c", h=H),
                    in1=rec_t[:, :, None].to_broadcast([P, H, C]), op=OP.mult)
                nc.vector.tensor_tensor(out=sc_t[:], in0=sc_t[:], in1=brep_t[:], op=OP.add)
                pos_t = fin_pool.tile([P, HC], F32, tag="pos")
                nc.vector.tensor_scalar(out=pos_t[:], in0=sc_t[:], scalar1=0.0,
                                        scalar2=None, op0=OP.max)
                nc.vector.tensor_scalar(out=sc_t[:], in0=sc_t[:], scalar1=0.0,
                                        scalar2=None, op0=OP.min)
                ex_t = fin_pool.tile([P, HC], F32, tag="expm")
                nc.scalar.activation(ex_t[:], sc_t[:], AF.Exp)
                ob_t = fin_pool.tile([P, HC + 1], BF16, tag="ob")
                nc.vector.scalar_tensor_tensor(out=ob_t[:, 0:HC], in0=ex_t[:],
                                               scalar=-1.0, in1=pos_t[:],
                                               op0=OP.add, op1=OP.add)
                if li < 2:
                    # write transposed activations for next layer's dense phase
                    for half in range(2):
                        tp = pt.tile([P, P], BF16, tag="tp")
                        nc.tensor.transpose(out=tp[0:P, 0:wr],
                                            in_=ob_t[0:wr, half * P:(half + 1) * P],
                                            identity=idb_t[0:wr, 0:wr])
                        xo_t = fin_pool.tile([P, P], BF16, tag="xo")
                        nc.vector.tensor_copy(out=xo_t[:, 0:wr], in_=tp[:, 0:wr])
                        nc.sync.dma_start(out=xT_own[half * P:(half + 1) * P, base:base + wr],
                                          in_=xo_t[:, 0:wr])
                else:
                    # graph mean-pool: indicator matmul, accumulated over windows
                    # garbage rows (>= wr, and trash slot 127) are finite and
                    # nullified by the indicator's zero rows -- no masking
                    nc.vector.memset(ob_t[:, HC:HC + 1], 1.0)
                    b_t = edg.tile([P, G], BF16, tag="bind")
                    nc.sync.dma_start(out=b_t[:], in_=bind_in[w])
                    nc.tensor.matmul(out=pool_ps[:], lhsT=b_t[:], rhs=ob_t[:],
                                     start=(w == 0), stop=(w == NWIN - 1))

            if li < 2:
                nc.gpsimd.collective_compute(
                    "AllGather", OP.bypass,
                    replica_groups=[list(range(NCORES))],
                    ins=[xT_own[:]], outs=[xT_all[:]])

        # ---------- pooling reduce + MLP (replicated on every core)
        pl_t = fin_pool.tile([G, HC + 1], F32, tag="pl")
        nc.vector.tensor_copy(out=pl_t[:], in_=pool_ps[:])
        nc.sync.dma_start(out=pool_in[:], in_=pl_t[:])
        nc.gpsimd.collective_compute(
            "AllReduce", OP.add, replica_groups=[list(range(NCORES))],
            ins=[pool_in[:]], outs=[pool_out[:]])
        gsum_t = fin_pool.tile([G, HC + 1], F32, tag="gsum")
        nc.sync.dma_start(out=gsum_t[:], in_=pool_out[:])
        cnt_r = sml.tile([G, 1], F32, tag="cntr")
        nc.vector.reciprocal(out=cnt_r[:], in_=gsum_t[:, HC:HC + 1])
        g_bf = fin_pool.tile([G, HC], BF16, tag="gbf")
        nc.vector.tensor_scalar(out=g_bf[:], in0=gsum_t[:, 0:HC], scalar1=cnt_r[:],
                                scalar2=None, op0=OP.mult)
        # g^T
        gT = []
        for half in range(2):
            tp = pt.tile([P, G], BF16, tag="tp")
            nc.tensor.transpose(out=tp[:], in_=g_bf[:, half * P:(half + 1) * P],
                                identity=idb_t[0:G, 0:G])
            gT_t = sml.tile([P, G], BF16, tag="gT")
            nc.vector.tensor_copy(out=gT_t[:], in_=tp[:])
            gT.append(gT_t)
        wm1 = []
        for half in range(2):
            wm1_t = sml.tile([P, C], BF16, tag="wm1")
            nc.gpsimd.dma_start(out=wm1_t[:], in_=pr['Wm1'][half * P:(half + 1) * P, :])
            wm1.append(wm1_t)
        ps1 = pt.tile([G, C], F32, tag="tp")
        for half in range(2):
            nc.tensor.matmul(out=ps1[:], lhsT=gT[half][:], rhs=wm1[half][:],
                             start=(half == 0), stop=(half == 1))
        bm1_row = sml.tile([1, C], F32, tag="bm1row")
        nc.sync.dma_start(out=bm1_row[:], in_=pr['bm1'][None, :])
        bm1_r = sml.tile([G, C], F32, tag="bm1r")
        nc.gpsimd.partition_broadcast(bm1_r[:], bm1_row[:])
        r1_t = sml.tile([G, C], F32, tag="r1")
        nc.vector.tensor_tensor(out=r1_t[:], in0=ps1[:], in1=bm1_r[:], op=OP.add)
        r1b_t = sml.tile([G, C], BF16, tag="r1b")
        nc.vector.tensor_scalar(out=r1b_t[:], in0=r1_t[:], scalar1=0.0,
                                scalar2=None, op0=OP.max)
        tp2 = pt.tile([C, G], BF16, tag="tp")
        nc.tensor.transpose(out=tp2[:], in_=r1b_t[:], identity=idb_t[0:G, 0:G])
        r1T_t = sml.tile([C, G], BF16, tag="r1T")
        nc.vector.tensor_copy(out=r1T_t[:], in_=tp2[:])
        wm2_t = sml.tile([C, 2], BF16, tag="wm2")
        nc.gpsimd.dma_start(out=wm2_t[:], in_=pr['Wm2'][:])
        ps2 = pt.tile([G, 2], F32, tag="tp")
        nc.tensor.matmul(out=ps2[:], lhsT=r1T_t[:], rhs=wm2_t[:], start=True, stop=True)
        bm2_row = sml.tile([1, 2], F32, tag="bm2row")
        nc.sync.dma_start(out=bm2_row[:], in_=pr['bm2'][None, :])
        bm2_r = sml.tile([G, 2], F32, tag="bm2r")
        nc.gpsimd.partition_broadcast(bm2_r[:], bm2_row[:])
        o_t = sml.tile([G, 2], F32, tag="ofin")
        nc.vector.tensor_tensor(out=o_t[:], in0=ps2[:], in1=bm2_r[:], op=OP.add)
        nc.sync.dma_start(out=out_p[:], in_=o_t[:])

    nc.finalize()
    return nc


# ---------------------------------------------------------------- execution

_CACHE = {}


def _get_program(N, NLOC, NWIN, NCW, G, FIN0):
    key = (N, NLOC, NWIN, NCW, G, FIN0)
    if key not in _CACHE:
        _CACHE[key] = build_program(N, NLOC, NWIN, NCW, G, FIN0)
    return _CACHE[key]


def _static_inputs(FIN0):
    iota = np.tile(np.arange(P, dtype=np.float32), (P, 1)).astype(NPBF16)
    identf = np.eye(P, dtype=np.float32)
    identb = np.eye(P, dtype=np.float32).astype(NPBF16)
    maskblk = np.zeros((P, 2, H), np.float32)
    for pg in range(2 * P):
        maskblk[pg % P, pg // P, pg // C] = 1.0
    return dict(iota=iota, identf=identf, identb=identb, maskblk=maskblk)


def kernel(**inputs):
    x = np.asarray(inputs['x'], np.float32)
    N, FIN0 = x.shape
    G = 64
    data, NLOC, NWIN, NCW = _build_host_data(
        inputs['edge_index'], inputs['batch'], N, G)
    nc = _get_program(N, NLOC, NWIN, NCW, G, FIN0)

    x0tb = np.ascontiguousarray(
        x.T.reshape(FIN0, NCORES, NLOC).transpose(1, 0, 2))
    common = dict(x0tb=x0tb, **_static_inputs(FIN0))
    for nm in ('W0', 'as0', 'ad0', 'b0', 'W1', 'as1', 'ad1', 'b1',
               'W2', 'as2', 'ad2', 'b2', 'Wm1', 'bm1', 'Wm2', 'bm2'):
        common[nm] = np.asarray(inputs[nm], np.float32)

    in_maps = [{**common, **data[k]} for k in range(NCORES)]
    res = run_bass_kernel_spmd(nc, in_maps, list(range(NCORES)))
    return np.asarray(res.results[0]['out'], np.float32)


# revision 16
# speedup vs baseline: 1.7306x; 1.7306x over previous
"""GAT (3-layer, 4-head) + mean-pool + MLP on 8 Trainium2 NeuronCores.

Strategy (see spec sharding_hint):
  - Nodes sharded 8 ways by destination; each core owns N/8 dst nodes and all
    edges incident to them (1-D graph partition, built on host from the index
    tensors only).
  - Dense phase replicated: every core computes the full node table
    T[n] = [h(256) | alpha_src(4) | alpha_dst(4)] in bf16 via matmuls whose
    rhs carries two extra score columns (Ws = W @ blockdiag(a_s), Wd likewise).
  - Edge phase: per 128-edge chunk, dma_gather h/alpha rows by src (and alpha
    rows by dst), compute al = exp(leaky_relu(as+ad)) (softmax shift skipped --
    mathematically invariant), then scatter-accumulate al*h into a PSUM window
    of 127 dst slots via a selection-matrix matmul; the matmul also sums al
    into 4 extra columns, yielding the softmax denominators for free.
  - Between layers: AllGather of the (transposed) activations; final graph
    mean-pool via indicator matmul accumulated across windows + AllReduce.
"""
import math
import numpy as np
from contextlib import ExitStack

import concourse.bass as bass
import concourse.bacc as bacc
import concourse.mybir as mybir
import concourse.tile as tile
from concourse import library_config
from concourse.bass_utils import run_bass_kernel_spmd

F32 = mybir.dt.float32
BF16 = mybir.dt.bfloat16
I16 = mybir.dt.int16
NPBF16 = mybir.dt.np(BF16)

P = 128
WIN = 127          # real dst slots per PSUM window; slot 127 collects padding
NCORES = 8
H, C = 4, 64
HC = H * C         # 256
NEG = 0.2
TSTRIDE = 384      # table row stride (elements); bf16 -> 768B (%256 == 0)

AF = mybir.ActivationFunctionType
OP = mybir.AluOpType


# ----------------------------------------------------------------- host prep

def _wrap_idx(idx_flat):
    """[num] -> [128, num//16] int16 tile layout: index j at [j%16, j//16],
    replicated 8x along partitions (one stripe per Q7 core)."""
    num = idx_flat.shape[0]
    w = idx_flat.reshape(num // 16, 16).T.astype(np.int16)   # [16, num//16]
    return np.tile(w, (8, 1))


def _prep_core(k, src, dst, batch, N, NLOC, NWIN, G):
    """Per-core edge structure. Returns dict of window-major arrays + counts."""
    sel = (dst >= k * NLOC) & (dst < (k + 1) * NLOC)
    s = src[sel].astype(np.int64)
    dl = (dst[sel] - k * NLOC).astype(np.int64)
    order = np.argsort(dl, kind="stable")
    s, dl = s[order], dl[order]
    w = dl // WIN
    slot = dl - w * WIN
    counts = np.bincount(w, minlength=NWIN)
    return s, w, slot, counts


def _build_host_data(edge_index, batch, N, G):
    NLOC = N // NCORES
    NWIN = math.ceil(NLOC / WIN)
    src = np.concatenate([np.asarray(edge_index[0]), np.arange(N)])
    dst = np.concatenate([np.asarray(edge_index[1]), np.arange(N)])
    batch = np.asarray(batch)

    per_core = [_prep_core(k, src, dst, batch, N, NLOC, NWIN, G)
                for k in range(NCORES)]
    NCW = max(int(math.ceil(c / P)) for (_, _, _, cnts) in per_core
              for c in cnts)
    NE = NCW * P  # padded edges per window

    data = []
    NCW_ = NE // P
    for k in range(NCORES):
        s, w, slot, counts = per_core[k]
        hsrc = np.zeros((NWIN, NE), np.int64)          # gather-by-src rows
        slots = np.full((NWIN, NE), WIN, np.int64)     # pad -> trash slot 127
        off = np.concatenate([[0], np.cumsum(counts)])
        for wi in range(NWIN):
            cnt = counts[wi]
            sl = slice(off[wi], off[wi] + cnt)
            hsrc[wi, :cnt] = s[sl]
            slots[wi, :cnt] = slot[sl]
        hidx = np.stack([_wrap_idx(hsrc[wi]) for wi in range(NWIN)])
        # one-hot selection matrices (pure graph structure, host-built):
        #   smat[w, e, c, j] = 1 iff edge (c*128+e) of window w targets slot j
        #   mmat[w, j, c, e] = same, transposed (for the alpha_dst expansion)
        oh = np.zeros((NWIN, NCW_, P, P), NPBF16)
        wi_i, ce_i = np.meshgrid(np.arange(NWIN), np.arange(NE), indexing='ij')
        oh[wi_i, ce_i // P, ce_i % P, slots] = 1.0
        smat = np.ascontiguousarray(oh.transpose(0, 2, 1, 3))
        mmat = np.ascontiguousarray(oh.transpose(0, 3, 1, 2))
        bind = np.zeros((NWIN, P, G), np.float32)
        for wi in range(NWIN):
            base = k * NLOC + wi * WIN
            nreal = min(WIN, (k + 1) * NLOC - base)
            rows = np.arange(nreal)
            bind[wi, rows, batch[base + rows]] = 1.0
        adidx = np.stack([
            _wrap_idx(np.minimum(k * NLOC + wi * WIN + np.arange(P), N + P - 1))
            for wi in range(NWIN)])
        data.append(dict(
            hidx=hidx.astype(np.int16),
            adidx=adidx.astype(np.int16),
            smat=smat,
            mmat=mmat,
            bind=bind.astype(NPBF16),
        ))
    return data, NLOC, NWIN, NCW


# ------------------------------------------------------------ program build

def _strided_view(t_ap, offset, stride, rows, width):
    """AP over DRAM table: rows x width, row-stride `stride`, elem offset."""
    flat = t_ap.rearrange("v s -> (v s)")
    flat = flat[offset:offset + rows * stride]
    v = flat.rearrange("(v s) -> v s", s=stride)
    return v[:, 0:width]


def build_program(N, NLOC, NWIN, NCW, G, FIN0):
    """Builds the full 3-layer SPMD program. Same program for all cores."""
    NT = N + P           # table rows (+P zero pad rows: alpha_d window loads overrun)
    NE = NCW * P
    NBLK = NCORES        # node blocks (one per core's node range)
    NTIL = math.ceil(NLOC / P)   # dense tiles per block
    fins = [FIN0, HC, HC]

    nc = bacc.Bacc()

    # ---------- parameters
    pr = {}
    for i, fin in enumerate(fins):
        pr[f'W{i}'] = nc.declare_dram_parameter(f'W{i}', [fin, HC], F32, isOutput=False)
        pr[f'as{i}'] = nc.declare_dram_parameter(f'as{i}', [H, C], F32, isOutput=False)
        pr[f'ad{i}'] = nc.declare_dram_parameter(f'ad{i}', [H, C], F32, isOutput=False)
        pr[f'b{i}'] = nc.declare_dram_parameter(f'b{i}', [HC], F32, isOutput=False)
    pr['Wm1'] = nc.declare_dram_parameter('Wm1', [HC, C], F32, isOutput=False)
    pr['bm1'] = nc.declare_dram_parameter('bm1', [C], F32, isOutput=False)
    pr['Wm2'] = nc.declare_dram_parameter('Wm2', [C, 2], F32, isOutput=False)
    pr['bm2'] = nc.declare_dram_parameter('bm2', [2], F32, isOutput=False)
    x0tb = nc.declare_dram_parameter('x0tb', [NBLK, FIN0, NLOC], F32, isOutput=False)
    hidx_in = nc.declare_dram_parameter('hidx', [NWIN, P, NE // 16], I16, isOutput=False)
    smat_in = nc.declare_dram_parameter('smat', [NWIN, P, NCW, P], BF16, isOutput=False)
    adidx_in = nc.declare_dram_parameter('adidx', [NWIN, P, P // 16], I16, isOutput=False)
    mmat_in = nc.declare_dram_parameter('mmat', [NWIN, P, NCW, P], BF16, isOutput=False)
    bind_in = nc.declare_dram_parameter('bind', [NWIN, P, G], BF16, isOutput=False)
    idf_in = nc.declare_dram_parameter('identf', [P, P], F32, isOutput=False)
    idb_in = nc.declare_dram_parameter('identb', [P, P], BF16, isOutput=False)
    mblk_in = nc.declare_dram_parameter('maskblk', [P, 2, H], F32, isOutput=False)
    out_p = nc.declare_dram_parameter('out', [G, 2], F32, isOutput=True)

    # ---------- internal DRAM
    T = nc.dram_tensor('Tbl', [NT, TSTRIDE], BF16)
    xT_own = nc.dram_tensor('xT_own', [HC, NLOC], BF16)
    xT_all = nc.dram_tensor('xT_all', [NBLK, HC, NLOC], BF16, addr_space="Shared")
    pool_in = nc.dram_tensor('pool_in', [G, HC + 1], F32)
    pool_out = nc.dram_tensor('pool_out', [G, HC + 1], F32, addr_space="Shared")

    with ExitStack() as ctx:
        tc = ctx.enter_context(tile.TileContext(nc))
        cst = ctx.enter_context(tc.tile_pool(name="cst", bufs=1))
        dns = ctx.enter_context(tc.tile_pool(name="dns", bufs=2))
        wts = ctx.enter_context(tc.tile_pool(name="wts", bufs=2))
        edg = ctx.enter_context(tc.tile_pool(name="edg", bufs=2))
        sml = ctx.enter_context(tc.tile_pool(name="sml", bufs=4))
        fin_pool = ctx.enter_context(tc.tile_pool(name="fin", bufs=2))
        pd = ctx.enter_context(tc.tile_pool(name="pd", bufs=2, space="PSUM"))
        pa = ctx.enter_context(tc.tile_pool(name="pa", bufs=2, space="PSUM"))
        pt = ctx.enter_context(tc.tile_pool(name="pt", bufs=2, space="PSUM"))
        pp = ctx.enter_context(tc.tile_pool(name="pp", bufs=1, space="PSUM"))

        # constants
        idf_t = cst.tile([P, P], F32)
        nc.sync.dma_start(out=idf_t[:], in_=idf_in[:])
        idb_t = cst.tile([P, P], BF16)
        nc.sync.dma_start(out=idb_t[:], in_=idb_in[:])
        mblk_t = cst.tile([P, 2, H], F32)
        nc.sync.dma_start(out=mblk_t[:], in_=mblk_in[:])

        # zero the table once: gathers read full rows incl. pad columns
        zt = dns.tile([P, TSTRIDE], BF16, tag="zt")
        nc.vector.memset(zt[:], 0.0)
        for r0 in range(0, NT, P):
            wd = min(P, NT - r0)
            nc.sync.dma_start(out=T[r0:r0 + wd, :], in_=zt[0:wd, :])

        T_h_view = T[:]                                       # rows for h-gather

        pool_ps = pp.tile([G, HC + 1], F32)

        for li, fin in enumerate(fins):
            nk = fin // P    # k-tiles (1 or 2)

            # ---- per-layer weight prep: Wc[kt] = [W | Ws | Wd] in bf16
            wt = []
            for kt in range(nk):
                w_t = wts.tile([P, HC], F32, tag="wld")
                nc.sync.dma_start(out=w_t[:], in_=pr[f'W{li}'][kt * P:(kt + 1) * P, :])
                wt.append(w_t)
            # W^T tiles: WT[cb] = [128 (c-block), fin]
            WT = []
            for cb in range(2):
                wT_t = wts.tile([P, fin], F32, tag="wT")
                for kt in range(nk):
                    tp = pt.tile([P, P], F32, tag="tp")
                    nc.tensor.transpose(out=tp[:], in_=wt[kt][:, cb * P:(cb + 1) * P],
                                        identity=idf_t[:])
                    nc.vector.tensor_copy(out=wT_t[:, kt * P:(kt + 1) * P], in_=tp[:])
                WT.append(wT_t)
            # A-blocks for a_s / a_d  (A[p, h] = a[h, c]*[p//64 == h], p = 64h+c)
            ws_sb = []
            for which in ('as', 'ad'):
                a_flat = sml.tile([P, 1], F32, tag="aflat")
                acc = pt.tile([H, fin], F32, tag="tp")
                for cb in range(2):
                    nc.sync.dma_start(
                        out=a_flat[:],
                        in_=pr[f'{which}{li}'][:].rearrange("h c -> (h c)")[cb * P:(cb + 1) * P, None])
                    ab_t = sml.tile([P, H], F32, tag="ablk")
                    nc.vector.tensor_tensor(out=ab_t[:], in0=a_flat[:].to_broadcast([P, H]),
                                            in1=mblk_t[:, cb, :], op=OP.mult)
                    nc.tensor.matmul(out=acc[:], lhsT=ab_t[:], rhs=WT[cb][:],
                                     start=(cb == 0), stop=(cb == 1))
                wsT_sb = sml.tile([H, fin], F32, tag="wsTsb")
                nc.vector.tensor_copy(out=wsT_sb[:], in_=acc[:])
                ws_sb.append(wsT_sb)
            # assemble Wc tiles
            Wc = []
            for kt in range(nk):
                wc_t = wts.tile([P, HC + 2 * H], BF16, tag="wc")
                nc.vector.tensor_copy(out=wc_t[:, 0:HC], in_=wt[kt][:])
                for wi, wsT_sb in enumerate(ws_sb):
                    tp = pt.tile([P, H], F32, tag="tp")
                    nc.tensor.transpose(out=tp[:], in_=wsT_sb[:, kt * P:(kt + 1) * P],
                                        identity=idf_t[0:H, 0:H])
                    nc.vector.tensor_copy(
                        out=wc_t[:, HC + wi * H:HC + (wi + 1) * H], in_=tp[:])
                Wc.append(wc_t)
            # bias replicated
            b_row = sml.tile([1, HC], F32, tag="brow")
            nc.sync.dma_start(out=b_row[:], in_=pr[f'b{li}'][None, :])
            brep_t = cst.tile([P, HC], F32, tag="brep")
            nc.gpsimd.partition_broadcast(brep_t[:], b_row[:])

            # ---- dense phase: T[n] = [x @ Wc] for all n
            for blk in range(NBLK):
                xb = []
                for kt in range(nk):
                    xb_t = dns.tile([P, NLOC], BF16, tag=f"xb{kt}")
                    if li == 0:
                        nc.gpsimd.dma_start(out=xb_t[:], in_=x0tb[blk, kt * P:(kt + 1) * P, :])
                    else:
                        nc.sync.dma_start(out=xb_t[:], in_=xT_all[blk, kt * P:(kt + 1) * P, :])
                    xb.append(xb_t)
                for t in range(NTIL):
                    lo = t * P
                    wd = min(P, NLOC - lo)
                    ps_t = pd.tile([P, HC + 2 * H], F32, tag="pdense")
                    for kt in range(nk):
                        nc.tensor.matmul(out=ps_t[0:wd, :], lhsT=xb[kt][:, lo:lo + wd],
                                         rhs=Wc[kt][:], start=(kt == 0), stop=(kt == nk - 1))
                    tb_t = dns.tile([P, HC + 2 * H], BF16, tag="tb")
                    nc.scalar.copy(out=tb_t[0:wd, :], in_=ps_t[0:wd, :])
                    row0 = blk * NLOC + lo
                    nc.sync.dma_start(out=T[row0:row0 + wd, 0:HC + 2 * H], in_=tb_t[0:wd, :])

            # ---- edge phase (this core's windows)
            for w in range(NWIN):
                base = w * WIN
                wr = min(WIN, NLOC - base)
                hix = edg.tile([P, NE // 16], I16, tag="hix")
                nc.sync.dma_start(out=hix[:], in_=hidx_in[w])
                sm_t = edg.tile([P, NCW, P], BF16, tag="sm")
                nc.sync.dma_start(out=sm_t[:], in_=smat_in[w])
                mm_t = edg.tile([P, NCW, P], BF16, tag="mm")
                nc.sync.dma_start(out=mm_t[:], in_=mmat_in[w])
                # alpha_dst rows for this window's slots (tiny per-core gather:
                # SPMD program can't address per-core rows statically)
                aix = edg.tile([P, P // 16], I16, tag="aix")
                nc.sync.dma_start(out=aix[:], in_=adidx_in[w])
                adr = edg.tile([P, 1, TSTRIDE], BF16, tag="adr")
                nc.gpsimd.dma_gather(out_ap=adr[:], in_ap=T_h_view, idxs_ap=aix[:],
                                     num_idxs=P, num_idxs_reg=P,
                                     elem_size=TSTRIDE, elem_step=TSTRIDE)
                ad_t = adr[:, 0, HC + H:HC + 2 * H]

                gh = edg.tile([P, NCW, TSTRIDE], BF16, tag="gh")
                nc.gpsimd.dma_gather(out_ap=gh[:], in_ap=T_h_view, idxs_ap=hix[:],
                                     num_idxs=NE, num_idxs_reg=NE,
                                     elem_size=TSTRIDE, elem_step=TSTRIDE,
                                     single_packet=False)

                agg = pa.tile([P, HC + H], F32, tag="agg")
                for c in range(NCW):
                    adp = pt.tile([P, H], F32, tag="tp")
                    nc.tensor.matmul(out=adp[:], lhsT=mm_t[:, c, :], rhs=ad_t,
                                     start=True, stop=True)
                    e_t = sml.tile([P, H], F32, tag="e")
                    nc.vector.tensor_tensor(out=e_t[:], in0=gh[:, c, HC:HC + H],
                                            in1=adp[:], op=OP.add)
                    l_t = sml.tile([P, H], F32, tag="lrelu")
                    nc.vector.scalar_tensor_tensor(out=l_t[:], in0=e_t[:], scalar=NEG,
                                                   in1=e_t[:], op0=OP.mult, op1=OP.max)
                    rhs_t = sml.tile([P, HC + H], BF16, tag="rhs")
                    nc.scalar.activation(rhs_t[:, HC:HC + H], l_t[:], AF.Exp)
                    nc.vector.tensor_tensor(
                        out=rhs_t[:, 0:HC].rearrange("p (h c) -> p h c", h=H),
                        in0=gh[:, c, 0:HC].rearrange("p (h c) -> p h c", h=H),
                        in1=rhs_t[:, HC:HC + H, None].to_broadcast([P, H, C]),
                        op=OP.mult)
                    nc.tensor.matmul(out=agg[:], lhsT=sm_t[:, c, :], rhs=rhs_t[:],
                                     start=(c == 0), stop=(c == NCW - 1))

                # finalize window: out = elu(num/den + b)
                rec_t = sml.tile([P, H], F32, tag="rec")
                nc.vector.tensor_scalar(out=rec_t[:], in0=agg[:, HC:HC + H],
                                        scalar1=1e-30, scalar2=None, op0=OP.add)
                nc.vector.reciprocal(out=rec_t[:], in_=rec_t[:])
                sc_t = fin_pool.tile([P, HC], F32, tag="scaled")
                nc.vector.tensor_tensor(
                    out=sc_t[:].rearrange("p (h c) -> p h c", h=H),
                    in0=agg[:, 0:HC].rearrange("p (h c) -> p h c", h=H),
                    in1=rec_t[:, :, None].to_broadcast([P, H, C]), op=OP.mult)
                nc.vector.tensor_tensor(out=sc_t[:], in0=sc_t[:], in1=brep_t[:], op=OP.add)
                pos_t = fin_pool.tile([P, HC], F32, tag="pos")
                nc.vector.tensor_scalar(out=pos_t[:], in0=sc_t[:], scalar1=0.0,
                                        scalar2=None, op0=OP.max)
                nc.vector.tensor_scalar(out=sc_t[:], in0=sc_t[:], scalar1=0.0,
                                        scalar2=None, op0=OP.min)
                ex_t = fin_pool.tile([P, HC], F32, tag="expm")
                nc.scalar.activation(ex_t[:], sc_t[:], AF.Exp)
                ob_t = fin_pool.tile([P, HC + 1], BF16, tag="ob")
                nc.vector.scalar_tensor_tensor(out=ob_t[:, 0:HC], in0=ex_t[:],
                                               scalar=-1.0, in1=pos_t[:],
                                               op0=OP.add, op1=OP.add)
                if li < 2:
                    # write transposed activations for next layer's dense phase
                    for half in range(2):
                        tp = pt.tile([P, P], BF16, tag="tp")
                        nc.tensor.transpose(out=tp[0:P, 0:wr],
                                            in_=ob_t[0:wr, half * P:(half + 1) * P],
                                            identity=idb_t[0:wr, 0:wr])
                        xo_t = fin_pool.tile([P, P], BF16, tag="xo")
                        nc.vector.tensor_copy(out=xo_t[:, 0:wr], in_=tp[:, 0:wr])
                        nc.sync.dma_start(out=xT_own[half * P:(half + 1) * P, base:base + wr],
                                          in_=xo_t[:, 0:wr])
                else:
                    # graph mean-pool: indicator matmul, accumulated over windows
                    # garbage rows (>= wr, and trash slot 127) are finite and
                    # nullified by the indicator's zero rows -- no masking
                    nc.vector.memset(ob_t[:, HC:HC + 1], 1.0)
                    b_t = edg.tile([P, G], BF16, tag="bind")
                    nc.sync.dma_start(out=b_t[:], in_=bind_in[w])
                    nc.tensor.matmul(out=pool_ps[:], lhsT=b_t[:], rhs=ob_t[:],
                                     start=(w == 0), stop=(w == NWIN - 1))

            if li < 2:
                nc.gpsimd.collective_compute(
                    "AllGather", OP.bypass,
                    replica_groups=[list(range(NCORES))],
                    ins=[xT_own[:]], outs=[xT_all[:]])

        # ---------- pooling reduce + MLP (replicated on every core)
        pl_t = fin_pool.tile([G, HC + 1], F32, tag="pl")
        nc.vector.tensor_copy(out=pl_t[:], in_=pool_ps[:])
        nc.sync.dma_start(out=pool_in[:], in_=pl_t[:])
        nc.gpsimd.collective_compute(
            "AllReduce", OP.add, replica_groups=[list(range(NCORES))],
            ins=[pool_in[:]], outs=[pool_out[:]])
        gsum_t = fin_pool.tile([G, HC + 1], F32, tag="gsum")
        nc.sync.dma_start(out=gsum_t[:], in_=pool_out[:])
        cnt_r = sml.tile([G, 1], F32, tag="cntr")
        nc.vector.reciprocal(out=cnt_r[:], in_=gsum_t[:, HC:HC + 1])
        g_bf = fin_pool.tile([G, HC], BF16, tag="gbf")
        nc.vector.tensor_scalar(out=g_bf[:], in0=gsum_t[:, 0:HC], scalar1=cnt_r[:],
                                scalar2=None, op0=OP.mult)
        # g^T
        gT = []
        for half in range(2):
            tp = pt.tile([P, G], BF16, tag="tp")
            nc.tensor.transpose(out=tp[:], in_=g_bf[:, half * P:(half + 1) * P],
                                identity=idb_t[0:G, 0:G])
            gT_t = sml.tile([P, G], BF16, tag="gT")
            nc.vector.tensor_copy(out=gT_t[:], in_=tp[:])
            gT.append(gT_t)
        wm1 = []
        for half in range(2):
            wm1_t = sml.tile([P, C], BF16, tag="wm1")
            nc.gpsimd.dma_start(out=wm1_t[:], in_=pr['Wm1'][half * P:(half + 1) * P, :])
            wm1.append(wm1_t)
        ps1 = pt.tile([G, C], F32, tag="tp")
        for half in range(2):
            nc.tensor.matmul(out=ps1[:], lhsT=gT[half][:], rhs=wm1[half][:],
                             start=(half == 0), stop=(half == 1))
        bm1_row = sml.tile([1, C], F32, tag="bm1row")
        nc.sync.dma_start(out=bm1_row[:], in_=pr['bm1'][None, :])
        bm1_r = sml.tile([G, C], F32, tag="bm1r")
        nc.gpsimd.partition_broadcast(bm1_r[:], bm1_row[:])
        r1_t = sml.tile([G, C], F32, tag="r1")
        nc.vector.tensor_tensor(out=r1_t[:], in0=ps1[:], in1=bm1_r[:], op=OP.add)
        r1b_t = sml.tile([G, C], BF16, tag="r1b")
        nc.vector.tensor_scalar(out=r1b_t[:], in0=r1_t[:], scalar1=0.0,
                                scalar2=None, op0=OP.max)
        tp2 = pt.tile([C, G], BF16, tag="tp")
        nc.tensor.transpose(out=tp2[:], in_=r1b_t[:], identity=idb_t[0:G, 0:G])
        r1T_t = sml.tile([C, G], BF16, tag="r1T")
        nc.vector.tensor_copy(out=r1T_t[:], in_=tp2[:])
        wm2_t = sml.tile([C, 2], BF16, tag="wm2")
        nc.gpsimd.dma_start(out=wm2_t[:], in_=pr['Wm2'][:])
        ps2 = pt.tile([G, 2], F32, tag="tp")
        nc.tensor.matmul(out=ps2[:], lhsT=r1T_t[:], rhs=wm2_t[:], start=True, stop=True)
        bm2_row = sml.tile([1, 2], F32, tag="bm2row")
        nc.sync.dma_start(out=bm2_row[:], in_=pr['bm2'][None, :])
        bm2_r = sml.tile([G, 2], F32, tag="bm2r")
        nc.gpsimd.partition_broadcast(bm2_r[:], bm2_row[:])
        o_t = sml.tile([G, 2], F32, tag="ofin")
        nc.vector.tensor_tensor(out=o_t[:], in0=ps2[:], in1=bm2_r[:], op=OP.add)
        nc.sync.dma_start(out=out_p[:], in_=o_t[:])

    nc.finalize()
    return nc


# ---------------------------------------------------------------- execution

_CACHE = {}


def _get_program(N, NLOC, NWIN, NCW, G, FIN0):
    key = (N, NLOC, NWIN, NCW, G, FIN0)
    if key not in _CACHE:
        _CACHE[key] = build_program(N, NLOC, NWIN, NCW, G, FIN0)
    return _CACHE[key]


def _static_inputs(FIN0):
    identf = np.eye(P, dtype=np.float32)
    identb = np.eye(P, dtype=np.float32).astype(NPBF16)
    maskblk = np.zeros((P, 2, H), np.float32)
    for pg in range(2 * P):
        maskblk[pg % P, pg // P, pg // C] = 1.0
    return dict(identf=identf, identb=identb, maskblk=maskblk)


def kernel(**inputs):
    x = np.asarray(inputs['x'], np.float32)
    N, FIN0 = x.shape
    G = 64
    data, NLOC, NWIN, NCW = _build_host_data(
        inputs['edge_index'], inputs['batch'], N, G)
    nc = _get_program(N, NLOC, NWIN, NCW, G, FIN0)

    x0tb = np.ascontiguousarray(
        x.T.reshape(FIN0, NCORES, NLOC).transpose(1, 0, 2))
    common = dict(x0tb=x0tb, **_static_inputs(FIN0))
    for nm in ('W0', 'as0', 'ad0', 'b0', 'W1', 'as1', 'ad1', 'b1',
               'W2', 'as2', 'ad2', 'b2', 'Wm1', 'bm1', 'Wm2', 'bm2'):
        common[nm] = np.asarray(inputs[nm], np.float32)

    in_maps = [{**common, **data[k]} for k in range(NCORES)]
    res = run_bass_kernel_spmd(nc, in_maps, list(range(NCORES)))
    return np.asarray(res.results[0]['out'], np.float32)
